# revision 10
# baseline (speedup 1.0000x reference)
"""Causal single-head attention (N=4096, din=dout=4096) on 8 TRN2 NeuronCores.

Math (reference):
    q = x @ Wq.T ; k = x @ Wk.T ; v = x @ Wv.T
    scores = q @ k.T ; keep j >= i (triu), else -inf
    out = softmax(scores / sqrt(N)) @ v

Reformulation (associativity moves the sharding point so each core computes
only its own 512 output rows with zero redundant FLOPs and no collectives):
    scores = (x Wq.T) Wk x.T          -> per-core: q = xr @ Wq.T ; t = q @ Wk ;
                                          scores_rows = t @ x.T
    out = (attn @ x) @ Wv.T           -> per-core: u = attn @ x ; out = u @ Wv.T

All five matmuls are arranged "TN" (contraction on the partition axis for both
operands) by host-side pre-transposes of x, Wq, Wv; Wk is used as stored.

Causal-triangle work skipping with a *uniform* instruction stream across the 8
SPMD cores: core c owns 256-row chunks c (slot P) and 8+c (slot Q).  Its xT /
xw inputs are the host-shifted (by 256*c) and zero-padded transposed/natural x,
so the j-tile loop bounds are core-independent; out-of-range j-tiles read zeros
and are killed by a per-tile -30000 bias folded into the exp() activation.
Slot Q's j-tiles coincide with the second half of slot P's shifted range, so
one shifted array serves both slots.

Matmuls run in float32r (reduced-precision fp32, 1 cycle/row at free-dim>=256)
with fp32 PSUM accumulation.
"""

import sys

sys.path.insert(0, "/opt/trn_rl_repo")

from contextlib import ExitStack

import numpy as np

from concourse import bacc, bass, mybir
from concourse.bass_utils import run_bass_kernel_spmd
from concourse.tile import TileContext

F32 = mybir.dt.float32
F32R = mybir.dt.float32r
EXP = mybir.ActivationFunctionType.Exp
NEG = -30000.0
P = 128


def build_nc(N, ncores):
    """One SPMD module; all per-core variation comes through the inputs."""
    NT = N // P          # number of 128-wide tiles along any axis
    KP = NT              # slot P j-tile slots
    KQ = NT // 2         # slot Q j-tile slots
    HOFF = KP // 2       # merged-loop k at which slot Q work begins
    OG = N // 512        # 512-wide output column groups
    scale = 1.0 / float(np.sqrt(N))

    nc = bacc.Bacc("TRN2", target_bir_lowering=False)
    d_xrT = nc.declare_dram_parameter("xrT", [N, 512], F32R, isOutput=False)
    d_xT = nc.declare_dram_parameter("xT", [N, N], F32R, isOutput=False)
    d_xw = nc.declare_dram_parameter("xw", [N, N], F32R, isOutput=False)
    d_wqT = nc.declare_dram_parameter("wqT", [N, N], F32R, isOutput=False)
    d_wk = nc.declare_dram_parameter("wk", [N, N], F32R, isOutput=False)
    d_wvT = nc.declare_dram_parameter("wvT", [N, N], F32R, isOutput=False)
    d_jb = nc.declare_dram_parameter("jbias", [P, KP + KQ], F32, isOutput=False)
    d_m0 = nc.declare_dram_parameter("mask0", [P, 256], F32R, isOutput=False)
    d_ones = nc.declare_dram_parameter("ones", [P, P], F32R, isOutput=False)
    d_m1 = nc.declare_dram_parameter("mask1", [P, 256], F32R, isOutput=False)
    d_out = nc.declare_dram_parameter("out", [512, N], F32, isOutput=True)

    with nc.allow_low_precision(reason="fp32r tiles; fp32 PSUM accumulation throughout"), TileContext(nc) as tc:
        with ExitStack() as ctx:
            const = ctx.enter_context(tc.tile_pool(name="const", bufs=1))
            ones_t = const.tile([P, P], F32R)
            nc.sync.dma_start(out=ones_t[:], in_=d_ones[:, :])
            jb_t = const.tile([P, KP + KQ], F32)
            nc.sync.dma_start(out=jb_t[:], in_=d_jb[:, :])
            m0_t = const.tile([P, 256], F32R)
            nc.sync.dma_start(out=m0_t[:], in_=d_m0[:, :])
            m1_t = const.tile([P, 256], F32R)
            nc.sync.dma_start(out=m1_t[:], in_=d_m1[:, :])

            # --- step 1: qT[o, i] = (xr @ Wq.T).T ------------------------
            cm_xr = tc.tile_pool(name="xr", bufs=1, side="left")
            p_xr = cm_xr.__enter__()
            xr_t = p_xr.tile([P, NT, 512], F32R)
            nc.sync.dma_start(
                out=xr_t[:], in_=d_xrT[:, :].rearrange("(t p) i -> p t i", p=P)
            )
            cm_q = tc.tile_pool(name="qT", bufs=1, side="right")
            p_q = cm_q.__enter__()
            qT_t = p_q.tile([P, NT, 512], F32R)

            with tc.tile_pool(name="wq_s", bufs=4) as p_wq, tc.tile_pool(
                name="ps1", bufs=8, space="PSUM"
            ) as p_ps1:
                for og in range(OG):
                    pss = [p_ps1.tile([P, 512], F32, tag="ps1", name=f"ps1_{og}_{i}") for i in range(4)]
                    for dt in range(NT):
                        ws = p_wq.tile([P, 512], F32R, tag="ws")
                        nc.sync.dma_start(
                            out=ws[:],
                            in_=d_wqT[P * dt : P * (dt + 1), 512 * og : 512 * (og + 1)],
                        )
                        for o4 in range(4):
                            nc.tensor.matmul(
                                pss[o4][:],
                                lhsT=(ws[:, P * o4 : P * (o4 + 1)]),
                                rhs=(xr_t[:, dt, :]),
                                start=(dt == 0),
                                stop=(dt == NT - 1),
                            )
                    for o4 in range(4):
                        nc.vector.tensor_copy(out=qT_t[:, 4 * og + o4, :], in_=pss[o4][:])
            cm_xr.__exit__(None, None, None)

            # --- step 2: tT[d', i] = (q @ Wk).T --------------------------
            cm_t = tc.tile_pool(name="tT", bufs=1, side="left")
            p_t = cm_t.__enter__()
            tT_t = p_t.tile([P, NT, 512], F32R)

            with tc.tile_pool(name="wk_s", bufs=4) as p_wk, tc.tile_pool(
                name="ps2", bufs=8, space="PSUM"
            ) as p_ps2:
                for dg in range(OG):
                    pss = [p_ps2.tile([P, 512], F32, tag="ps2", name=f"ps2_{dg}_{i}") for i in range(4)]
                    for ot in range(NT):
                        ws = p_wk.tile([P, 512], F32R, tag="wks")
                        nc.sync.dma_start(
                            out=ws[:],
                            in_=d_wk[P * ot : P * (ot + 1), 512 * dg : 512 * (dg + 1)],
                        )
                        for d4 in range(4):
                            nc.tensor.matmul(
                                pss[d4][:],
                                lhsT=(ws[:, P * d4 : P * (d4 + 1)]),
                                rhs=(qT_t[:, ot, :]),
                                start=(ot == 0),
                                stop=(ot == NT - 1),
                            )
                    for d4 in range(4):
                        nc.vector.tensor_copy(out=tT_t[:, 4 * dg + d4, :], in_=pss[d4][:])
            cm_q.__exit__(None, None, None)

            # --- step 3: scoresT[j, i] per j-tile; exp; mask; denom ------
            cm_a = tc.tile_pool(name="attn", bufs=1, side="right")
            p_a = cm_a.__enter__()
            attnP = p_a.tile([P, KP, 256], F32R)
            attnQ = p_a.tile([P, KQ, 256], F32R)
            recP = p_a.tile([P, 256], F32R)
            recQ = p_a.tile([P, 256], F32R)

            with tc.tile_pool(name="xc", bufs=3) as p_xc, tc.tile_pool(
                name="ps3", bufs=4, space="PSUM"
            ) as p_ps3, tc.tile_pool(name="psd", bufs=2, space="PSUM") as p_psd:
                for k in range(KP):
                    xc = p_xc.tile([P, NT, P], F32R, tag="xc")
                    nc.sync.dma_start(
                        out=xc[:],
                        in_=d_xT[:, P * k : P * (k + 1)].rearrange(
                            "(t p) j -> p t j", p=P
                        ),
                    )
                    psP = p_ps3.tile([P, 256], F32, tag="ps3")
                    for dt in range(NT):
                        nc.tensor.matmul(
                            psP[:],
                            lhsT=(xc[:, dt, :]),
                            rhs=(tT_t[:, dt, 0:256]),
                            start=(dt == 0),
                            stop=(dt == NT - 1),
                        )
                    nc.scalar.activation(
                        attnP[:, k, :], psP[:], EXP, bias=jb_t[:, k : k + 1], scale=scale
                    )
                    if k == 0:
                        nc.vector.tensor_mul(attnP[:, 0, :], attnP[:, 0, :], m0_t[:])
                    elif k == 1:
                        nc.vector.tensor_mul(attnP[:, 1, :], attnP[:, 1, :], m1_t[:])
                    if k >= HOFF:
                        kq = k - HOFF
                        psQ = p_ps3.tile([P, 256], F32, tag="ps3")
                        for dt in range(NT):
                            nc.tensor.matmul(
                                psQ[:],
                                lhsT=(xc[:, dt, :]),
                                rhs=(tT_t[:, dt, 256:512]),
                                start=(dt == 0),
                                stop=(dt == NT - 1),
                            )
                        nc.scalar.activation(
                            attnQ[:, kq, :],
                            psQ[:],
                            EXP,
                            bias=jb_t[:, KP + kq : KP + kq + 1],
                            scale=scale,
                        )
                        if kq == 0:
                            nc.vector.tensor_mul(attnQ[:, 0, :], attnQ[:, 0, :], m0_t[:])
                        elif kq == 1:
                            nc.vector.tensor_mul(attnQ[:, 1, :], attnQ[:, 1, :], m1_t[:])

                # softmax denominators: ones-matrix matmul broadcasts the
                # column sums to every output partition.
                psdP = p_psd.tile([P, 256], F32, tag="psd")
                for k in range(KP):
                    nc.tensor.matmul(
                        psdP[:],
                        lhsT=(ones_t[:]),
                        rhs=(attnP[:, k, :]),
                        start=(k == 0),
                        stop=(k == KP - 1),
                    )
                nc.vector.reciprocal(recP[:], psdP[:])
                psdQ = p_psd.tile([P, 256], F32, tag="psd")
                for kq in range(KQ):
                    nc.tensor.matmul(
                        psdQ[:],
                        lhsT=(ones_t[:]),
                        rhs=(attnQ[:, kq, :]),
                        start=(kq == 0),
                        stop=(kq == KQ - 1),
                    )
                nc.vector.reciprocal(recQ[:], psdQ[:])
                for k in range(KP):
                    nc.vector.tensor_mul(attnP[:, k, :], attnP[:, k, :], recP[:])
                for kq in range(KQ):
                    nc.vector.tensor_mul(attnQ[:, kq, :], attnQ[:, kq, :], recQ[:])
            cm_t.__exit__(None, None, None)

            # --- step 5: uT[d, i] = (attn @ x).T -------------------------
            cm_u = tc.tile_pool(name="uT", bufs=1, side="left")
            p_u = cm_u.__enter__()
            uT_t = p_u.tile([P, NT, 512], F32R)

            with tc.tile_pool(name="xwc", bufs=3) as p_xw, tc.tile_pool(
                name="ps5", bufs=4, space="PSUM"
            ) as p_ps5:
                for dt in range(NT):
                    xwc = p_xw.tile([P, KP, P], F32R, tag="xwc")
                    nc.sync.dma_start(
                        out=xwc[:],
                        in_=d_xw[:, P * dt : P * (dt + 1)].rearrange(
                            "(t p) d -> p t d", p=P
                        ),
                    )
                    psuP = p_ps5.tile([P, 256], F32, tag="ps5")
                    for k in range(KP):
                        nc.tensor.matmul(
                            psuP[:],
                            lhsT=(xwc[:, k, :]),
                            rhs=(attnP[:, k, :]),
                            start=(k == 0),
                            stop=(k == KP - 1),
                        )
                    psuQ = p_ps5.tile([P, 256], F32, tag="ps5")
                    for kq in range(KQ):
                        nc.tensor.matmul(
                            psuQ[:],
                            lhsT=(xwc[:, HOFF + kq, :]),
                            rhs=(attnQ[:, kq, :]),
                            start=(kq == 0),
                            stop=(kq == KQ - 1),
                        )
                    nc.vector.tensor_copy(out=uT_t[:, dt, 0:256], in_=psuP[:])
                    nc.vector.tensor_copy(out=uT_t[:, dt, 256:512], in_=psuQ[:])
            cm_a.__exit__(None, None, None)

            # --- step 6: out[i, o] = u @ Wv.T ----------------------------
            with tc.tile_pool(name="wv_s", bufs=4) as p_wv, tc.tile_pool(
                name="ps6", bufs=8, space="PSUM"
            ) as p_ps6, tc.tile_pool(name="ob", bufs=4) as p_ob:
                for og in range(OG):
                    pss = [p_ps6.tile([P, 512], F32, tag="ps6", name=f"ps6_{og}_{i}") for i in range(4)]
                    for dt in range(NT):
                        vs = p_wv.tile([P, 512], F32R, tag="vs")
                        nc.sync.dma_start(
                            out=vs[:],
                            in_=d_wvT[P * dt : P * (dt + 1), 512 * og : 512 * (og + 1)],
                        )
                        for it in range(4):
                            nc.tensor.matmul(
                                pss[it][:],
                                lhsT=(uT_t[:, dt, P * it : P * (it + 1)]),
                                rhs=(vs[:]),
                                start=(dt == 0),
                                stop=(dt == NT - 1),
                            )
                    for it in range(4):
                        ob = p_ob.tile([P, 512], F32, tag="ob")
                        nc.vector.tensor_copy(out=ob[:], in_=pss[it][:])
                        nc.sync.dma_start(
                            out=d_out[P * it : P * (it + 1), 512 * og : 512 * (og + 1)],
                            in_=ob[:],
                        )
            cm_u.__exit__(None, None, None)
    nc.finalize()
    return nc


def host_inputs(x, Wq, Wk, Wv, ncores):
    N = x.shape[0]
    pad = 256 * (ncores - 1)
    KP = N // P
    KQ = KP // 2

    xT = np.zeros((N, N + pad), np.float32)
    xT[:, :N] = x.T
    xw = np.zeros((N + pad, N), np.float32)
    xw[:N, :] = x
    wqT = np.ascontiguousarray(Wq.T)
    wvT = np.ascontiguousarray(Wv.T)
    wk = np.ascontiguousarray(Wk)

    jj = np.arange(P)[:, None]
    ii = np.arange(256)[None, :]
    m0 = (jj >= ii).astype(np.float32)
    m1 = ((jj + P) >= ii).astype(np.float32)

    in_maps = []
    for c in range(ncores):
        s = 256 * c
        xrT = np.concatenate(
            [xT[:, s : s + 256], xT[:, N // 2 + s : N // 2 + s + 256]], axis=1
        )
        jb = np.zeros((P, KP + KQ), np.float32)
        jb[:, KP - 2 * c : KP] = NEG
        jb[:, KP + KQ - 2 * c :] = NEG
        in_maps.append(
            {
                "xrT": np.ascontiguousarray(xrT),
                "xT": np.ascontiguousarray(xT[:, s : s + N]),
                "xw": np.ascontiguousarray(xw[s : s + N, :]),
                "wqT": wqT,
                "wk": wk,
                "wvT": wvT,
                "jbias": jb,
                "mask0": m0,
                "ones": np.ones((P, P), np.float32),
                "mask1": m1,
            }
        )
    return in_maps


def gather_out(results, N, ncores):
    out = np.empty((N, N), np.float32)
    for c in range(ncores):
        s = 256 * c
        out[s : s + 256] = results[c]["out"][:256]
        out[N // 2 + s : N // 2 + s + 256] = results[c]["out"][256:]
    return out


_NC_CACHE = {}


def run(x, Wq, Wk, Wv, ncores=None, trace=False, **spmd_kwargs):
    x = np.ascontiguousarray(np.asarray(x, dtype=np.float32))
    Wq = np.asarray(Wq, dtype=np.float32)
    Wk = np.asarray(Wk, dtype=np.float32)
    Wv = np.asarray(Wv, dtype=np.float32)
    N = x.shape[0]
    if ncores is None:
        ncores = N // 512
    key = (N, ncores)
    if key not in _NC_CACHE:
        _NC_CACHE[key] = build_nc(N, ncores)
    nc = _NC_CACHE[key]
    in_maps = host_inputs(x, Wq, Wk, Wv, ncores)
    br = run_bass_kernel_spmd(
        nc, in_maps, list(range(ncores)), trace=trace, **spmd_kwargs
    )
    return gather_out(br.results, N, ncores), br


def kernel(x, Wq, Wk, Wv):
    out, _ = run(x, Wq, Wk, Wv)
    return out
